# revision 14
# baseline (speedup 1.0000x reference)
"""Adaptive linear (per-batch expert weight gather + matmul + bias) on 8 TRN2 cores.

Reference semantics:
    out[b, n, o] = sum_k x[b, n, k] * weight[indices[b], k, o] + bias[indices[b], 0, o]
with x [256, 1024, 256], indices [256], weight [1024, 256, 256], bias [1024, 1, 256].

Sharding: data-parallel over the batch dim B=256 -> 32 batches per core. The
weight/bias tables are replicated to every core; each core gathers the 32
weight tiles it needs ON DEVICE, driven by its indices (dynamic-offset DMAs
whose base address comes from a register loaded from the indices tensor),
then runs bf16 matmuls (w stationary, x moving) accumulating in fp32 PSUM,
adds the gathered bias during the PSUM drain, and writes out.

Engine plan per core:
  - sync   (HWDGE): 32 dynamic weight gathers (reg = idx value loaded from
           SBUF; in_ = wtab[ds(reg*128, 128), :])
  - gpsimd (SWDGE): x loads with inline f32->bf16 cast (SWDGE-only feature),
           plus the bias indirect gather
  - tensor: bf16 matmuls, K split in two 128-partition PSUM-accumulated chunks
  - vector: weight f32->bf16 rounding + PSUM drain (f=0 half) with bias add
  - scalar: PSUM drain (f=1 half) with bias add + output stores (HWDGE ring)

Layout choices (host-side, pure layout/sharding transforms):
  - x is passed per-core transposed with the contraction dim (IN) on SBUF
    partitions, interleaved even/odd: x_t[p, j, b, n] = x[b, n, 2p+j]. This
    matches the packed weight layout so no on-device transpose is needed; the
    contraction splits into two K=128 chunks (j=0: even k, j=1: odd k).
  - the weight table is passed as rows [C*128, 512]: row (c*128+p) holds
    weight[c, 2p:2p+2, :]. One dynamic DMA per batch (128 rows of 2KB) pulls
    w[indices[b]] into SBUF in exactly the lhsT layout.
  - output is produced as out^T ([OUT, BL*N], bf16) and upcast/transposed back
    on the host after gathering.
"""

import numpy as np

from concourse import bacc, bass, mybir, tile
from concourse.bass_utils import run_bass_kernel_spmd
from concourse.masks import make_identity

NCORES = 8
B, N, IN, OUT, C = 256, 1024, 256, 256, 1024
BL = B // NCORES          # 32 batches per core
KC = 2                    # contraction chunks (even/odd interleave planes)
MC = OUT // 128           # 2 output-partition chunks
FD = 512                  # max matmul free dim into one fp32 PSUM bank
FC = N // FD              # 2 free chunks
NB = 4                    # batches per x/out DMA group

COMPUTE = "bf16"          # "bf16" (fast, ~1e-3 rel err) or "f32" (exact)
OUT_BF16 = True           # write out^T as bf16 (halves store traffic)
GATHER_MODE = "indirect"  # "dyn_sync" (dynamic HWDGE) or "indirect" (gpsimd)
XLOAD_MODE = "swdge_cast" # "swdge_cast" (gpsimd, casts to bf16) or "hwdge_f32"

_F32 = mybir.dt.float32
_BF16 = mybir.dt.bfloat16
_I32 = mybir.dt.int32

_nc_cache = []
_last_in_maps = None


def _build():
    nc = bacc.Bacc("TRN2", target_bir_lowering=False, debug=False, num_devices=NCORES)
    x_t = nc.dram_tensor("x_t", [128, KC * BL * N], _F32, kind="ExternalInput").ap()
    wtab = nc.dram_tensor("wtab", [C * 128, KC * OUT], _F32, kind="ExternalInput").ap()
    btab = nc.dram_tensor("btab", [C, OUT], _F32, kind="ExternalInput").ap()
    woff = nc.dram_tensor("woff", [128, BL], _I32, kind="ExternalInput").ap()
    idx = nc.dram_tensor("idx", [BL], _I32, kind="ExternalInput").ap()
    idxrow = nc.dram_tensor("idxrow", [1, BL], _I32, kind="ExternalInput").ap()
    out_t = nc.dram_tensor(
        "out_t", [OUT, BL * N], _BF16 if OUT_BF16 else _F32, kind="ExternalOutput"
    ).ap()

    bf16 = COMPUTE == "bf16"
    o_dt = _BF16 if OUT_BF16 else _F32

    with tile.TileContext(nc) as tc:
        with (
            tc.tile_pool(name="sb", bufs=1) as sb,
            tc.tile_pool(name="wp", bufs=1) as wp,
            tc.tile_pool(name="xp", bufs=1) as xp,
            tc.tile_pool(name="op", bufs=1) as op,
            tc.tile_pool(name="psp", bufs=1, space="PSUM") as psp,
        ):
            idxt = sb.tile([BL, 1], _I32, tag="idxt", bufs=1)
            nc.sync.dma_start(idxt[:], idx[0:BL, None])
            idxr = sb.tile([1, BL], _I32, tag="idxr", bufs=1)
            nc.sync.dma_start(idxr[:], idxrow[:])
            offs = sb.tile([128, BL], _I32, tag="offs", bufs=1)
            if GATHER_MODE == "indirect":
                nc.sync.dma_start(offs[:], woff[:])

            # bias: gather the 32 rows, then PE-transpose to [OUT-chunk, BL]
            ident = sb.tile([128, 128], _F32, tag="ident", bufs=1)
            make_identity(nc, ident[:])
            bsb = sb.tile([BL, OUT], _F32, tag="bsb", bufs=1)
            nc.gpsimd.indirect_dma_start(
                out=bsb[:],
                out_offset=None,
                in_=btab[:, :],
                in_offset=bass.IndirectOffsetOnAxis(ap=idxt[:, :1], axis=0),
            )
            bt = []
            for mc in range(MC):
                pst = psp.tile([128, FD], _F32, tag="mm", bufs=8, name=f"pst_{mc}")
                nc.tensor.transpose(
                    out=pst[:, :BL],
                    in_=bsb[:BL, mc * 128 : (mc + 1) * 128],
                    identity=ident[:BL, :BL],
                )
                btile = sb.tile([128, BL], _F32, tag="bt", bufs=2, name=f"bt_{mc}")
                nc.vector.tensor_copy(btile[:], pst[:, :BL])
                bt.append(btile)

            # gather all per-batch weight tiles up front (one DMA per batch;
            # row p of the gather = weight[idx, 2p:2p+2, :], 2KB), then round
            # each staged f32 tile to a resident bf16 copy on DVE.
            wt = []
            for b in range(BL):
                w = wp.tile(
                    [128, KC * OUT], _F32, tag="w",
                    bufs=(6 if bf16 else BL), name=f"w_{b}",
                )
                if GATHER_MODE == "dyn_sync":
                    v = nc.sync.value_load(
                        idxr[0:1, b : b + 1], min_val=0, max_val=C - 1
                    )
                    nc.sync.dma_start(w[:], wtab[bass.ds(v * 128, 128), :])
                else:
                    nc.gpsimd.indirect_dma_start(
                        out=w[:],
                        out_offset=None,
                        in_=wtab[:, :],
                        in_offset=bass.IndirectOffsetOnAxis(
                            ap=offs[:, b : b + 1], axis=0
                        ),
                    )
                if bf16:
                    wr = wp.tile(
                        [128, KC * OUT], _BF16, tag="wr", bufs=BL, name=f"wr_{b}"
                    )
                    nc.vector.tensor_copy(wr[:], w[:])
                    wt.append(wr)
                else:
                    wt.append(w)

            for bg in range(0, BL, NB):
                xs = []
                for j in range(KC):
                    src = x_t[:, (j * BL + bg) * N : (j * BL + bg + NB) * N]
                    if bf16 and XLOAD_MODE == "swdge_cast":
                        xr = xp.tile(
                            [128, NB * N], _BF16, tag=f"xr{j}", bufs=2,
                            name=f"xr_{bg}_{j}",
                        )
                        nc.gpsimd.dma_start(xr[:], src)
                        xs.append(xr)
                    else:
                        xt_ = xp.tile(
                            [128, NB * N], _F32, tag=f"x{j}", bufs=2,
                            name=f"x_{bg}_{j}",
                        )
                        nc.sync.dma_start(xt_[:], src)
                        if bf16:
                            xr = xp.tile(
                                [128, NB * N], _BF16, tag=f"xr{j}", bufs=2,
                                name=f"xr_{bg}_{j}",
                            )
                            nc.vector.tensor_copy(xr[:], xt_[:])
                            xs.append(xr)
                        else:
                            xs.append(xt_)
                os_ = []
                for mc in range(MC):
                    ot = op.tile(
                        [128, NB * N], o_dt, tag=f"o{mc}", bufs=2, name=f"o_{bg}_{mc}"
                    )
                    os_.append(ot)
                for j in range(NB):
                    b = bg + j
                    for mc in range(MC):
                        pss = []
                        for f in range(FC):
                            ps_mm = psp.tile(
                                [128, FD], _F32, tag="mm", bufs=8, name=f"mm_{b}_{mc}_{f}"
                            )
                            pss.append(ps_mm)
                        for kc in range(KC):
                            lhsT = wt[b][:, kc * OUT + mc * 128 : kc * OUT + (mc + 1) * 128]
                            for f in range(FC):
                                rhs = xs[kc][:, j * N + f * FD : j * N + (f + 1) * FD]
                                nc.tensor.matmul(
                                    pss[f][:],
                                    lhsT,
                                    rhs,
                                    start=(kc == 0),
                                    stop=(kc == KC - 1),
                                )
                        # drain+bias: f=0 on DVE, f=1 on ACT
                        nc.vector.tensor_tensor(
                            out=os_[mc][:, j * N + 0 * FD : j * N + 1 * FD],
                            in0=pss[0][:],
                            in1=bt[mc][:, b : b + 1].to_broadcast([128, FD]),
                            op=mybir.AluOpType.add,
                        )
                        nc.scalar.activation(
                            os_[mc][:, j * N + 1 * FD : j * N + 2 * FD],
                            pss[1][:],
                            mybir.ActivationFunctionType.Identity,
                            bias=bt[mc][:, b : b + 1],
                        )
                for mc in range(MC):
                    nc.scalar.dma_start(
                        out_t[mc * 128 : (mc + 1) * 128, bg * N : (bg + NB) * N],
                        os_[mc][:],
                    )

    nc.compile()
    return nc


def _get_nc():
    if not _nc_cache:
        _nc_cache.append(_build())
    return _nc_cache[0]


def kernel(x, indices, weight, bias):
    x = np.asarray(x, dtype=np.float32)
    idx_np = np.asarray(indices).astype(np.int64).reshape(B)
    # weight rows packed 2 IN-rows per row: row (c*128+p) = weight[c, 2p:2p+2, :]
    wtab = np.ascontiguousarray(np.asarray(weight, dtype=np.float32)).reshape(
        C * 128, KC * OUT
    )
    btab = np.ascontiguousarray(np.asarray(bias, dtype=np.float32)).reshape(C, OUT)

    nc = _get_nc()

    in_maps = []
    for c in range(NCORES):
        sl = slice(c * BL, (c + 1) * BL)
        # x_t[p, j, b, n] = x[b, n, 2p+j]
        xs = np.ascontiguousarray(
            np.transpose(x[sl].reshape(BL, N, 128, KC), (2, 3, 0, 1))
        ).reshape(128, KC * BL * N)
        il = idx_np[sl].astype(np.int32)
        woff = (
            il[None, :] * 128 + np.arange(128, dtype=np.int32)[:, None]
        ).astype(np.int32)
        in_maps.append(
            {
                "x_t": xs,
                "wtab": wtab,
                "btab": btab,
                "woff": woff,
                "idx": il,
                "idxrow": il.reshape(1, BL).copy(),
            }
        )

    global _last_in_maps
    _last_in_maps = in_maps

    res = run_bass_kernel_spmd(nc, in_maps, core_ids=list(range(NCORES)))

    outs = []
    for c in range(NCORES):
        ot = np.asarray(res.results[c]["out_t"], dtype=np.float32).reshape(OUT, BL, N)
        outs.append(np.transpose(ot, (1, 2, 0)))
    return np.ascontiguousarray(np.concatenate(outs, axis=0))
